# revision 1
# baseline (speedup 1.0000x reference)
"""Trainium2 Bass kernel for nn_EntanglementTransform.

Computes, for x[B,Q,H] and W[Q,Q,H]:
    factor[k,h] = prod_{j>k} W[k,j,h] * prod_{i<k} W[i,k,h]
    y = x * factor ;  out = y / max(||y||_2(axis=H), 1e-12)

Sharding over 8 NeuronCores:
  - x / out: data-parallel over batch (32 batches per core)
  - W: sharded over H (256 columns per core). Each core computes its
    factor[:, h-shard] in log-domain (sign tracked separately) via a
    masked-matmul pair-sum on the PE, then a tiny AllGather (64KB/core)
    assembles the full [Q, H] factor everywhere.

Only the Q*(Q-1)/2 = 2016 upper-triangle pairs (i<j) contribute, so the
host packs just those rows (padded to 2048) — halving W traffic and PE
work. The log magnitudes are split into bf16 hi+lo and packed side by
side so one N=512 bf16 matmul per K-chunk accumulates both halves; the
two PSUM column halves are recombined with one DVE add.

The log-domain product (exp of summed logs) reproduces f32 underflow
semantics: products below ~1e-45 come out as exactly 0, matching the
f32 reference.
"""

import os

os.environ.setdefault("MYCRO_LOCAL_CACHE", "1")

import numpy as np

N_CORES = 8
B, Q, H = 256, 64, 2048
BS = B // N_CORES          # 32 batches per core
HC = H // N_CORES          # 256 h-columns per core
R = BS * Q                 # 2048 (b,q) rows per core
NPAIR = Q * (Q - 1) // 2   # 2016 upper-triangle pairs
NW = 16                    # padded pair rows = NW*128 = 2048
W_CHUNKS = 8
TPC = NW // W_CHUNKS       # 4 row-tiles per chunk
NT = R // 128              # 16 x-tiles per core
EPS = 1e-12
LOG_BIAS = 1e-38           # ln(w^2 + bias): keeps ln finite at w == 0

_CACHE = {}


def _pair_index():
    """Row r enumerates pair (i, j) with i < j, row-major."""
    ii, jj = np.triu_indices(Q, k=1)
    return ii, jj


def _pair_mask():
    """mask[r, k] = 1.0 iff pair r = (i, j) touches k (k == i or k == j).

    Column k selects exactly the 63 pairs whose product forms factor[k].
    Rows NPAIR..NW*128 are zero padding.
    """
    ii, jj = _pair_index()
    m = np.zeros((NW * 128, Q), dtype=np.float32)
    r = np.arange(NPAIR)
    m[r, ii] = 1.0
    m[r, jj] = 1.0
    return m


def _swizzle_rows(a):
    """[T*128, F] row-major -> [128, T*F] with tile t at cols [t*F,(t+1)*F).

    Makes every per-tile DMA read fully contiguous per partition.
    """
    n, f = a.shape
    t = n // 128
    return np.ascontiguousarray(
        a.reshape(t, 128, f).transpose(1, 0, 2).reshape(128, t * f)
    )


def _build_module():
    import concourse.bacc as bacc
    import concourse.mybir as mybir
    from concourse import tile

    fp32 = mybir.dt.float32
    bf16 = mybir.dt.bfloat16
    ALU = mybir.AluOpType
    ACT = mybir.ActivationFunctionType

    nc = bacc.Bacc(None, num_devices=N_CORES, num_swdge_queues=4)

    xs = nc.declare_dram_parameter("xs", [R, H], fp32, isOutput=False)
    ws = nc.declare_dram_parameter("ws", [128, NW * HC], fp32, isOutput=False)
    mk16 = nc.declare_dram_parameter("mk16", [128, NW * Q], bf16, isOutput=False)
    out = nc.declare_dram_parameter("out", [R, H], fp32, isOutput=True)

    fac_local = nc.dram_tensor("fac_local", [Q, HC], fp32)
    fac_ag = nc.dram_tensor("fac_ag", [N_CORES, Q, HC], fp32, addr_space="Shared")
    warm_in = nc.dram_tensor("warm_in", [1, 1], fp32)
    warm_out = nc.dram_tensor("warm_out", [N_CORES, 1], fp32, addr_space="Shared")

    with tile.TileContext(nc, num_cores=N_CORES) as tc:
        with (
            tc.tile_pool(name="consts", bufs=1) as constp,
            tc.tile_pool(name="facp", bufs=1) as facp,
            tc.tile_pool(name="small", bufs=10) as smallp,
            tc.tile_pool(name="xp", bufs=13) as xp,
            tc.tile_pool(name="yp", bufs=6) as yp,
        ):
            mk16_sb = constp.tile([128, NW * Q], bf16, tag="mk16")
            f_sb = facp.tile([128, H], fp32, tag="f")
            ln_bias = constp.tile([128, 1], fp32, tag="lnb")
            nc.vector.memset(ln_bias[:], LOG_BIAS)
            # tiny warmup collective: pre-pays RDH channel setup so the real
            # AllGather below executes quickly once the factor is ready
            nc.sync.dma_start(out=warm_in[:], in_=ln_bias[0:1, 0:1])
            nc.gpsimd.collective_compute(
                "AllGather",
                ALU.bypass,
                replica_groups=[list(range(N_CORES))],
                ins=[warm_in[:]],
                outs=[warm_out[:]],
            )
            nc.sync.dma_start(out=mk16_sb[:], in_=mk16[:])

            # ---------------- W stage: factor[:, h-shard] ----------------
            with (
                tc.tile_pool(name="wp", bufs=3) as wp,
                tc.tile_pool(name="wsmall", bufs=1) as wsmallp,
                tc.tile_pool(name="lp", bufs=2) as lp,
                tc.tile_pool(name="rtp", bufs=2) as rtp,
                tc.tile_pool(name="ngp", bufs=2) as ngp,
                tc.tile_pool(name="wpsum", bufs=1, space="PSUM") as pp,
            ):
                # psum_l column halves hold sum(mask*hi) | sum(mask*lo);
                # recombined after the chain by one DVE add
                psum_l = pp.tile([Q, 2 * HC], fp32, tag="psl")
                psum_n = pp.tile([Q, HC], fp32, tag="psn")
                for c in range(W_CHUNKS):
                    wt = wp.tile([128, TPC * HC], fp32, tag="wt")
                    nc.scalar.dma_start(
                        out=wt[:], in_=ws[:, c * TPC * HC : (c + 1) * TPC * HC]
                    )
                    lt = lp.tile([128, TPC * HC], fp32, tag="lt")
                    rt = rtp.tile([128, TPC * 2 * HC], bf16, tag="rt")
                    nt = ngp.tile([128, TPC * HC], bf16, tag="nt")
                    # lt = ln(w^2 + eps) = 2*ln|w|; rt = bf16 [hi | lo] per
                    # row-tile; nt = (w < 0)
                    nc.vector.tensor_tensor(
                        out=lt[:], in0=wt[:], in1=wt[:], op=ALU.mult
                    )
                    nc.scalar.activation(
                        out=lt[:], in_=lt[:], func=ACT.Ln, bias=ln_bias[:], scale=1.0
                    )
                    lt_v = lt[:].rearrange("p (t h) -> p t h", h=HC)
                    rt_v = rt[:].rearrange("p (t s) -> p t s", s=2 * HC)
                    rt_hi = rt_v[:, :, 0:HC]
                    rt_lo = rt_v[:, :, HC : 2 * HC]
                    nc.vector.tensor_copy(rt_hi, lt_v)
                    nc.vector.tensor_tensor(
                        out=rt_lo, in0=lt_v, in1=rt_hi, op=ALU.subtract
                    )
                    nc.vector.tensor_scalar(nt[:], wt[:], 0.0, None, ALU.is_lt)
                    for t in range(TPC):
                        g = c * TPC + t
                        mkg = mk16_sb[:, g * Q : (g + 1) * Q]
                        nc.tensor.matmul(
                            psum_l[:],
                            lhsT=mkg,
                            rhs=rt[:, t * 2 * HC : (t + 1) * 2 * HC],
                            start=(g == 0), stop=(g == NW - 1),
                        )
                        nc.tensor.matmul(
                            psum_n[:],
                            lhsT=mkg,
                            rhs=nt[:, t * HC : (t + 1) * HC],
                            start=(g == 0), stop=(g == NW - 1),
                        )
                # |factor| = exp(0.5 * (hi-sums + lo-sums)); sign from parity
                # of neg-count (mod-2 via binary subtraction ladder: the DVE
                # tensor_scalar ALU has no mod op).
                lsum = wsmallp.tile([Q, HC], fp32, tag="lsum")
                ltmp = wsmallp.tile([Q, HC], fp32, tag="ltmp")
                mag = wsmallp.tile([Q, HC], fp32, tag="mag")
                sgn = wsmallp.tile([Q, HC], fp32, tag="sgn")
                par = wsmallp.tile([Q, HC], fp32, tag="par")
                bit = wsmallp.tile([Q, HC], fp32, tag="bit")
                fac = wsmallp.tile([Q, HC], fp32, tag="fac")
                nc.scalar.copy(ltmp[:], psum_l[:, HC : 2 * HC])
                nc.vector.tensor_tensor(
                    out=lsum[:], in0=psum_l[:, 0:HC], in1=ltmp[:], op=ALU.add,
                )
                nc.scalar.activation(
                    out=mag[:], in_=lsum[:], func=ACT.Exp, scale=0.5
                )
                src = psum_n[:]
                for v in (32.0, 16.0, 8.0, 4.0, 2.0):
                    nc.vector.tensor_scalar(bit[:], src, v, None, ALU.is_ge)
                    nc.vector.scalar_tensor_tensor(
                        out=par[:], in0=bit[:], scalar=-v, in1=src,
                        op0=ALU.mult, op1=ALU.add,
                    )
                    src = par[:]
                # par in {0,1}; sgn = 1 - 2*par in {+1,-1}
                nc.vector.tensor_scalar(sgn[:], par[:], -2.0, 1.0, ALU.mult, ALU.add)
                nc.vector.tensor_tensor(out=fac[:], in0=sgn[:], in1=mag[:], op=ALU.mult)
                nc.sync.dma_start(out=fac_local[:], in_=fac[:])
                nc.gpsimd.collective_compute(
                    "AllGather",
                    ALU.bypass,
                    replica_groups=[list(range(N_CORES))],
                    ins=[fac_local[:]],
                    outs=[fac_ag[:]],
                )
                # Full factor, rows duplicated to all 128 partitions
                # (row p of an x-tile has q = p % 64).
                ag_v = fac_ag[:].rearrange("m k h -> k m h")
                nc.sync.dma_start(out=f_sb[0:Q, :], in_=ag_v)
                nc.scalar.dma_start(out=f_sb[Q : 2 * Q, :], in_=ag_v)

            # ---------------- x stage: scale + normalize ----------------
            for i in range(NT):
                xt = xp.tile([128, H], fp32, tag="xt")
                nc.sync.dma_start(out=xt[:], in_=xs[i * 128 : (i + 1) * 128, :])
                yt = yp.tile([128, H], fp32, tag="yt")
                nc.vector.tensor_tensor(
                    out=yt[:], in0=xt[:], in1=f_sb[:], op=ALU.mult
                )
                ss = smallp.tile([128, 1], fp32, tag="ss")
                # y^2 is a dead store: write it over the consumed x tile
                nc.scalar.activation(
                    out=xt[:], in_=yt[:], func=ACT.Square, accum_out=ss[:]
                )
                nrm = smallp.tile([128, 1], fp32, tag="nrm")
                inv = smallp.tile([128, 1], fp32, tag="inv")
                nc.scalar.activation(out=nrm[:], in_=ss[:], func=ACT.Sqrt)
                nc.vector.tensor_scalar(nrm[:], nrm[:], EPS, None, ALU.max)
                nc.vector.reciprocal(out=inv[:], in_=nrm[:])
                if i % 8 >= 3:
                    nc.vector.tensor_scalar(yt[:], yt[:], inv[:], None, ALU.mult)
                else:
                    nc.scalar.activation(
                        out=yt[:], in_=yt[:], func=ACT.Copy, scale=inv[:]
                    )
                nc.sync.dma_start(
                    out=out[i * 128 : (i + 1) * 128, :], in_=yt[:]
                )
    if not nc.is_finalized():
        nc.finalize()
    return nc


def _get_module():
    if "nc" not in _CACHE:
        _CACHE["nc"] = _build_module()
    return _CACHE["nc"]


def _make_in_maps(x, entanglement_weights):
    import ml_dtypes

    x = np.ascontiguousarray(x, dtype=np.float32)
    w = np.ascontiguousarray(entanglement_weights, dtype=np.float32)
    mk16_sw = _swizzle_rows(_pair_mask()).astype(ml_dtypes.bfloat16)
    ii, jj = _pair_index()
    in_maps = []
    for m in range(N_CORES):
        xsh = np.ascontiguousarray(x[m * BS : (m + 1) * BS]).reshape(R, H)
        wsh = w[:, :, m * HC : (m + 1) * HC]          # [Q, Q, HC]
        wp = np.ones((NW * 128, HC), dtype=np.float32)
        wp[:NPAIR] = wsh[ii, jj]                      # upper-triangle pairs
        in_maps.append(
            {
                "xs": xsh,
                "ws": _swizzle_rows(wp),
                "mk16": mk16_sw,
            }
        )
    return in_maps


def _run(x, entanglement_weights, trace=False):
    from concourse.bass_utils import run_bass_kernel_spmd

    nc = _get_module()
    in_maps = _make_in_maps(x, entanglement_weights)
    res = run_bass_kernel_spmd(
        nc, in_maps, core_ids=list(range(N_CORES)), trace=trace
    )
    parts = [
        np.asarray(res.results[m]["out"], dtype=np.float32).reshape(BS, Q, H)
        for m in range(N_CORES)
    ]
    return np.concatenate(parts, axis=0), res


def kernel(x, entanglement_weights):
    out, _ = _run(x, entanglement_weights)
    return out

